# revision 64
# baseline (speedup 1.0000x reference)
"""MoE multi-head attention Trainium2 kernel.

Problem: x:[B=2,S=2048,D=1024], Wq:[H=4,E=4,D,DH=256], Wk/Wv:[D,D], Wr:[H,E*DH,E]
  K/V = per-head projections of x; Q per (head, expert); full softmax attention
  per (b,h,e); router softmax over experts from concat of expert outputs;
  router-weighted combine -> out [B,S,H,DH].

Sharding: 8 cores = B*H (2 batches x 4 heads). Each core computes all E=4
experts for its (b,h) pair, so the router combine is fully core-local and no
collectives are needed.

Per-core pipeline (everything "transposed": features on SBUF partitions):
  P0: transpose x[b] -> xT [D, S] via PE transposes
  P1: K.T = Wk_h.T@ x.T, V = x@Wv_h (token-major), Q.T[e] -> DRAM scratch
  P2: per (s-tile, e): stream over t-chunks: scores.T = K.T^T-chunks @ Q.T,
      exp on ACT (scale=1/sqrt(DH), no max subtraction -- scores are O(1)),
      eo_u.T += V-chunk.T @ attn.T (PSUM accum), rowsum via ones-matmul.
  P3: router logits from eo_u.T (per-expert partials scaled by 1/rowsum),
      transpose logits to token-major, softmax over E=4 on DVE/ACT,
      transpose eo_u.T blocks and combine with w/rowsum as per-partition
      scalars, DMA out.

All matmul operands are float32r (full PE rate at N>=256; measured precision
~1.3e-4 scale-relative vs fp32).
"""
import sys

sys.path.insert(0, "/opt/trn_rl_repo")

import math

import numpy as np

import concourse.bass as bass
import concourse.mybir as mybir
import concourse.tile as tile
from concourse import bacc, bass_utils

B, S, D = 2, 2048, 1024
H, E, DH = 4, 4, 256
SCALE = math.sqrt(DH)
NCORES = B * H

DC = D // 128      # 8 contraction chunks over D
KC = DH // 128     # 2 chunks over head dim
ST = S // 512      # 4 tiles of 512 tokens
TT = S // 128      # 16 tiles of 128 tokens

F32 = mybir.dt.float32
F32R = mybir.dt.float32r

_cached = None
_last_in_maps = None


def _build(upto=3, p3parts="LRSC"):
    nc = bacc.Bacc("TRN2", target_bir_lowering=False, debug=False)

    x_d = nc.dram_tensor("x", [S, D], F32R, kind="ExternalInput")
    wk_d = nc.dram_tensor("wk", [128, DC * DH], F32R, kind="ExternalInput")
    wv_d = nc.dram_tensor("wv", [128, DC * DH], F32R, kind="ExternalInput")
    wq_d = nc.dram_tensor("wq", [128, E * DC * DH], F32R, kind="ExternalInput")
    wr_d = nc.dram_tensor("wr", [128, (E * KC) * E], F32R, kind="ExternalInput")
    id_r = nc.dram_tensor("id_r", [128, 128], F32R, kind="ExternalInput")
    id_f = nc.dram_tensor("id_f", [128, 128], F32, kind="ExternalInput")
    ones_d = nc.dram_tensor("ones", [128, 8], F32R, kind="ExternalInput")
    ones_f_d = nc.dram_tensor("ones_f", [128, 8], F32, kind="ExternalInput")
    out_d = nc.dram_tensor("out", [S, DH], F32, kind="ExternalOutput")
    if upto == 1:
        dbg_k = nc.dram_tensor("dbg_k", [128, KC * S], F32, kind="ExternalOutput")
        dbg_v = nc.dram_tensor("dbg_v", [128, TT * DH], F32, kind="ExternalOutput")
        dbg_q = nc.dram_tensor("dbg_q", [128, E * ST * KC * 512], F32, kind="ExternalOutput")
    if upto == 2:
        dbg_eo = nc.dram_tensor("dbg_eo", [128, E * KC * S], F32, kind="ExternalOutput")
        dbg_r = nc.dram_tensor("dbg_r", [128, 2 * S], F32, kind="ExternalOutput")

    with tile.TileContext(nc) as tc:
        with (
            tc.tile_pool(name="pw", bufs=1) as pw,
            tc.tile_pool(name="pdram", bufs=1, space="DRAM") as pdram,
            tc.tile_pool(name="pkv", bufs=1) as pkv,
        ):
            # ---- resident weights/constants ----
            wk_sb = pw.tile([128, DC * DH], F32R)
            wv_sb = pw.tile([128, DC * DH], F32R)
            wr_sb = pw.tile([128, (E * KC) * E], F32R)
            idr_sb = pw.tile([128, 128], F32R)
            idf_sb = pw.tile([128, 128], F32)
            ones_sb = pw.tile([128, 8], F32R)
            ones_f_sb = pw.tile([128, 8], F32)
            nc.scalar.dma_start(ones_f_sb[:], ones_f_d[:])
            nc.scalar.dma_start(wk_sb[:], wk_d[:])
            nc.scalar.dma_start(wv_sb[:], wv_d[:])
            nc.scalar.dma_start(wr_sb[:], wr_d[:])
            nc.scalar.dma_start(idr_sb[:], id_r[:])
            nc.scalar.dma_start(idf_sb[:], id_f[:])
            nc.scalar.dma_start(ones_sb[:], ones_d[:])

            k_sb = pkv.tile([128, KC * S], F32R)      # K.T  [k, (kc,t)]
            v_sb = pkv.tile([128, TT * DH], F32R)     # V    [t, (tt,k)]
            q_dram = pdram.tile([128, E * ST * KC * 512], F32R)

            # ================= Phase 0+1: transpose x; K,V,Q projections ====
            with (
                tc.tile_pool(name="pwq", bufs=1) as pwq,
                tc.tile_pool(name="px", bufs=3) as px,
                tc.tile_pool(name="pxT", bufs=1) as pxT,
                tc.tile_pool(name="pqst", bufs=4) as pqst,
                tc.tile_pool(name="ps_tr", bufs=3, space="PSUM") as ps_tr,
                tc.tile_pool(name="ps_p5", bufs=3, space="PSUM") as ps_p5,
                tc.tile_pool(name="ps_p2", bufs=2, space="PSUM") as ps_p2,
            ):
                wq_sb = pwq.tile([128, E * DC * DH], F32R)
                nc.scalar.dma_start(wq_sb[:], wq_d[:])
                xT = pxT.tile([128, DC * S], F32R)    # [d, (c, t)]
                for tt in range(TT):
                    x_t = px.tile([128, D], F32R, name="x_t")
                    nc.sync.dma_start(x_t[:], x_d[tt * 128:(tt + 1) * 128, :])
                    for c in range(DC):
                        tp = ps_tr.tile([128, 128], F32R, name="tp")
                        nc.tensor.transpose(tp[:], x_t[:, c * 128:(c + 1) * 128], idr_sb[:])
                        nc.vector.tensor_copy(xT[:, c * S + tt * 128:c * S + (tt + 1) * 128], tp[:])
                    # V tile tt only needs this x tile -- fills the DMA wait
                    vp = ps_p2.tile([128, DH], F32, name="vp")
                    for c in range(DC):
                        nc.tensor.matmul(
                            vp[:],
                            xT[:, c * S + tt * 128:c * S + (tt + 1) * 128],
                            wv_sb[:, c * DH:(c + 1) * DH],
                            start=(c == 0), stop=(c == DC - 1),
                        )
                    nc.vector.tensor_copy(v_sb[:, tt * DH:(tt + 1) * DH], vp[:])

                # K.T tiles [128k, 512t]
                for kc in range(KC):
                    for st in range(ST):
                        kp = ps_p5.tile([128, 512], F32, name="kp", tag="proj")
                        for c in range(DC):
                            nc.tensor.matmul(
                                kp[:],
                                wk_sb[:, c * DH + kc * 128:c * DH + (kc + 1) * 128],
                                xT[:, c * S + st * 512:c * S + (st + 1) * 512],
                                start=(c == 0), stop=(c == DC - 1),
                            )
                        nc.vector.tensor_copy(k_sb[:, kc * S + st * 512:kc * S + (st + 1) * 512], kp[:])

                # Q.T[e] tiles [128k, 512s] -> DRAM scratch [p,(e,st,kc,s)]
                for e in range(E):
                    for st in range(ST):
                        for kc in range(KC):
                            qp = ps_p5.tile([128, 512], F32, name="qp", tag="proj")
                            for c in range(DC):
                                nc.tensor.matmul(
                                    qp[:],
                                    wq_sb[:, (e * DC + c) * DH + kc * 128:(e * DC + c) * DH + (kc + 1) * 128],
                                    xT[:, c * S + st * 512:c * S + (st + 1) * 512],
                                    start=(c == 0), stop=(c == DC - 1),
                                )
                            qs = pqst.tile([128, 512], F32R, name="qs")
                            nc.vector.tensor_copy(qs[:], qp[:])
                            off = ((e * ST + st) * KC + kc) * 512
                            nc.sync.dma_start(q_dram[:, off:off + 512], qs[:])

            if upto == 1:
                nc.sync.dma_start(dbg_k[:], k_sb[:].bitcast(F32))
                nc.sync.dma_start(dbg_v[:], v_sb[:].bitcast(F32))
                nc.sync.dma_start(dbg_q[:], q_dram[:].bitcast(F32))

            with tc.tile_pool(name="peo", bufs=1) as peo:
                eo_sb = peo.tile([128, E * KC * S], F32R, name="eo_sb")
                # layout [k, (e, kc, s)] ; per (e,kc) slice is [128, S]
                # rowsums go to DRAM, then come back transposed via one
                # strided DMA (PE transposes of [1,128] rows crash here).
                r_dram = pdram.tile([4, S], F32, name="r_dram")

                def eo_slice(e, kc, lo, n):
                    base = (e * KC + kc) * S + lo
                    return eo_sb[:, base:base + n]

                # ===== Phases 2+3 fused per s-tile: attention, router, out ==
                # Phase-3 work for s-tile k overlaps phase-2 work for k+1;
                # all phase-3 PSUM tiles share one single-slot tag so the
                # PSUM budget stays at 8 banks (sc:2 eo:4 rp:1 p3:1).
                with (
                    tc.tile_pool(name="pql", bufs=2) as pql,
                    tc.tile_pool(name="pattn", bufs=6) as pattn,
                    tc.tile_pool(name="p3", bufs=2) as p3,
                    tc.tile_pool(name="pout", bufs=3) as pout,
                    tc.tile_pool(name="ps_sc", bufs=2, space="PSUM") as ps_sc,
                    tc.tile_pool(name="ps_eo", bufs=2, space="PSUM") as ps_eo,
                    tc.tile_pool(name="ps_r", bufs=1, space="PSUM") as ps_r,
                    tc.tile_pool(name="ps_p3", bufs=1, space="PSUM") as ps_p3,
                ):
                    rT = peo.tile([128, ST * 4 * E], F32, name="rT")
                    rTv = rT.rearrange("p (c e) -> p c e", e=E)
                    rrec = peo.tile([128, ST * 4 * E], F32, name="rrec")

                    for st in (range(ST) if upto >= 2 else ()):
                        # ---- attention for the 4 experts on this s-tile ----
                        for e in range(E):
                            ql = pql.tile([128, KC * 512], F32R, name="ql")
                            off = (e * ST + st) * KC * 512
                            nc.sync.dma_start(ql[:], q_dram[:, off:off + KC * 512])
                            eo0 = ps_eo.tile([128, 512], F32, name="eo0", tag="eo0")
                            eo1 = ps_eo.tile([128, 512], F32, name="eo1", tag="eo1")
                            eop = [eo0, eo1]
                            rp = ps_r.tile([1, 512], F32, name="rp")
                            for t in range(TT):
                                sc = ps_sc.tile([128, 512], F32, name="sc")
                                for kc in range(KC):
                                    nc.tensor.matmul(
                                        sc[:],
                                        k_sb[:, kc * S + t * 128:kc * S + (t + 1) * 128],
                                        ql[:, kc * 512:(kc + 1) * 512],
                                        start=(kc == 0), stop=(kc == KC - 1),
                                    )
                                at = pattn.tile([128, 512], F32R, name="at")
                                nc.scalar.activation(at[:], sc[:], mybir.ActivationFunctionType.Exp,
                                                     scale=1.0 / SCALE)
                                for kc in range(KC):
                                    nc.tensor.matmul(
                                        eop[kc][:],
                                        v_sb[:, t * DH + kc * 128:t * DH + (kc + 1) * 128],
                                        at[:],
                                        start=(t == 0), stop=(t == TT - 1),
                                    )
                                nc.tensor.matmul(
                                    rp[:], ones_sb[:, 0:1], at[:],
                                    start=(t == 0), stop=(t == TT - 1),
                                )
                            for kc in range(KC):
                                nc.vector.tensor_copy(eo_slice(e, kc, st * 512, 512), eop[kc][:])
                            rst = pattn.tile([1, 512], F32, name="rst", tag="rst")
                            nc.vector.tensor_copy(rst[:], rp[:])
                            nc.sync.dma_start(r_dram[e:e + 1, st * 512:(st + 1) * 512], rst[:])

                        if upto < 3:
                            continue

                        # ---- router + combine for this s-tile --------------
                        # transposed rowsums via DMA round trip (PE transposes
                        # of [1,128] rows crash the exec unit here)
                        for e in range(E):
                            nc.sync.dma_start(
                                rTv[:, st * 4:(st + 1) * 4, e:e + 1],
                                r_dram[e:e + 1, st * 512:(st + 1) * 512]
                                .rearrange("o (c p) -> p c o", p=128))
                        nc.vector.reciprocal(rrec[:, st * 16:(st + 1) * 16],
                                             rT[:, st * 16:(st + 1) * 16])

                        pls = []
                        for e in range(E):
                            pl = ps_p3.tile([4, 512], F32, name="pl", tag="p3s")
                            for kc in range(KC):
                                f = e * KC + kc
                                nc.tensor.matmul(
                                    pl[:],
                                    wr_sb[:, f * E:(f + 1) * E],
                                    eo_slice(e, kc, st * 512, 512),
                                    start=(kc == 0), stop=(kc == KC - 1),
                                )
                            pse = p3.tile([4, 512], F32, name=f"pls{e}", tag=f"pls{e}")
                            nc.vector.tensor_copy(pse[:], pl[:])
                            pls.append(pse)

                        for ss in range(4):
                            lo = st * 512 + ss * 128
                            rr = rrec[:, (st * 4 + ss) * E:(st * 4 + ss + 1) * E]
                            # logits [s, e'] = sum_e plT_e * (1/r_e[s])
                            lacc = p3.tile([128, 4], F32, name="lacc", tag="lacc")
                            for e in range(E):
                                plT = ps_p3.tile([128, 4], F32, name="plT", tag="p3s")
                                nc.tensor.transpose(plT[:], pls[e][:, ss * 128:(ss + 1) * 128],
                                                    idf_sb[0:4, 0:4])
                                if e == 0:
                                    nc.vector.tensor_scalar_mul(lacc[:], plT[:], rr[:, 0:1])
                                else:
                                    nc.vector.scalar_tensor_tensor(
                                        lacc[:], plT[:], rr[:, e:e + 1], lacc[:],
                                        mybir.AluOpType.mult, mybir.AluOpType.add,
                                    )
                            nmx = p3.tile([128, 1], F32, name="nmx", tag="nmx")
                            nc.vector.reduce_max(nmx[:], lacc[:], mybir.AxisListType.X, negate=True)
                            ex = p3.tile([128, 4], F32, name="ex", tag="ex")
                            sumx = p3.tile([128, 1], F32, name="sumx", tag="sumx")
                            nc.scalar.activation(ex[:], lacc[:], mybir.ActivationFunctionType.Exp,
                                                 bias=nmx[:], accum_out=sumx[:])
                            rw = p3.tile([128, 1], F32, name="rw", tag="rw")
                            nc.vector.reciprocal(rw[:], sumx[:])
                            w4 = p3.tile([128, 4], F32, name="w4", tag="w4")
                            nc.vector.tensor_scalar_mul(w4[:], ex[:], rw[:])
                            wn = p3.tile([128, 4], F32, name="wn", tag="wn")
                            nc.vector.tensor_tensor(wn[:], w4[:], rr[:], mybir.AluOpType.mult)

                            ob = pout.tile([128, DH], F32, name="ob")
                            for kc in range(KC):
                                for e in range(E):
                                    et = ps_p3.tile([128, 128], F32R, name="et", tag="p3s")
                                    nc.tensor.transpose(et[:], eo_slice(e, kc, lo, 128), idr_sb[:])
                                    dst = ob[:, kc * 128:(kc + 1) * 128]
                                    if e == 0:
                                        nc.vector.tensor_scalar_mul(dst, et[:], wn[:, 0:1])
                                    else:
                                        nc.vector.scalar_tensor_tensor(
                                            dst, et[:], wn[:, e:e + 1], dst,
                                            mybir.AluOpType.mult, mybir.AluOpType.add,
                                        )
                            nc.sync.dma_start(out_d[lo:lo + 128, :], ob[:])

                if upto == 2:
                    nc.sync.dma_start(dbg_eo[:], eo_sb[:].bitcast(F32))
                    nc.sync.dma_start(dbg_r[0:4, 0:S], r_dram[:])

    nc.compile()
    return nc


def _get_nc():
    global _cached
    if _cached is None:
        _cached = _build()
    return _cached


def kernel(x, Wq, Wk, Wv, Wr):
    global _last_in_maps
    x = np.asarray(x, dtype=np.float32)
    Wq = np.asarray(Wq, dtype=np.float32)
    Wk = np.asarray(Wk, dtype=np.float32)
    Wv = np.asarray(Wv, dtype=np.float32)
    Wr = np.asarray(Wr, dtype=np.float32)

    nc = _get_nc()

    ident = np.eye(128, dtype=np.float32)
    ones = np.ones((128, 8), dtype=np.float32)

    def chunked(w):  # [D, N] -> [128, DC*N] with layout [p, (c, n)]
        n = w.shape[1]
        return np.ascontiguousarray(w.reshape(DC, 128, n).transpose(1, 0, 2).reshape(128, DC * n))

    in_maps = []
    for c in range(NCORES):
        b, h = divmod(c, H)
        wq_h = Wq[h].reshape(E, DC, 128, DH).transpose(2, 0, 1, 3).reshape(128, E * DC * DH)
        wr_h = Wr[h].reshape(E * KC, 128, E).transpose(1, 0, 2).reshape(128, E * KC * E)
        in_maps.append({
            "x": np.ascontiguousarray(x[b]),
            "wk": chunked(Wk[:, h * DH:(h + 1) * DH]),
            "wv": chunked(Wv[:, h * DH:(h + 1) * DH]),
            "wq": np.ascontiguousarray(wq_h),
            "wr": np.ascontiguousarray(wr_h),
            "id_r": ident,
            "id_f": ident,
            "ones": ones,
            "ones_f": ones,
        })

    _last_in_maps = in_maps
    res = bass_utils.run_bass_kernel_spmd(nc, in_maps, core_ids=list(range(NCORES)))

    out = np.empty((B, S, H, DH), dtype=np.float32)
    for c in range(NCORES):
        b, h = divmod(c, H)
        out[b, :, h, :] = res.results[c]["out"]
    return out
